# revision 5
# baseline (speedup 1.0000x reference)
"""Multi-head attention (softmax over query axis) on 8 NeuronCores.

Sharding: core c -> batch c//4, head-pair c%4 (2 heads per core).
Host prep: transpose activations to [D, S], slice per-head weights.
On-chip: projections -> transposed scores K@Q^T -> exp (free-axis softmax
over q) with accum_out Z -> fold 1/Z into V -> attnV -> output projection,
all matmuls in fp32r. Host: sum head-pair partials per batch, transpose,
add output bias.
"""
import numpy as np
import concourse.bacc as bacc
import concourse.mybir as mybir
from concourse import tile
from concourse.bass_utils import run_bass_kernel_spmd

B, S, D = 2, 4096, 512
H, DK, DV = 8, 64, 64
HPC = 2                 # heads per core
NCORES = 8
PT = 128                # partition tile
KT = S // PT            # 32 key tiles
QB = 512                # q block (matmul free dim)
NQB = S // QB           # 8
DT = D // PT            # 4 d_in tiles
GRP = 4                 # k-tiles per attnV PSUM accumulation group
F32 = mybir.dt.float32
F32R = mybir.dt.float32r
EXP = mybir.ActivationFunctionType.Exp
AXX = mybir.AxisListType.X

_cached_nc = None


def _build():
    nc = bacc.Bacc("TRN2", target_bir_lowering=False, debug=False)

    xq_d = nc.dram_tensor("xq", [D, S], F32, kind="ExternalInput")
    xk_d = nc.dram_tensor("xk", [D, S], F32, kind="ExternalInput")
    xv_d = nc.dram_tensor("xv", [D, S], F32, kind="ExternalInput")
    wq_d = nc.dram_tensor("wq", [D, PT], F32, kind="ExternalInput")
    wk_d = nc.dram_tensor("wk", [D, PT], F32, kind="ExternalInput")
    wv_d = nc.dram_tensor("wv", [D, PT], F32, kind="ExternalInput")
    wo_d = nc.dram_tensor("wo", [PT, D], F32, kind="ExternalInput")
    bq_d = nc.dram_tensor("bq", [PT, 1], F32, kind="ExternalInput")
    bk_d = nc.dram_tensor("bk", [PT, 1], F32, kind="ExternalInput")
    bv_d = nc.dram_tensor("bv", [1, PT], F32, kind="ExternalInput")
    ones_d = nc.dram_tensor("ones", [1, PT], F32, kind="ExternalInput")
    out_d = nc.dram_tensor("outT", [D, S], F32, kind="ExternalOutput")

    with tile.TileContext(nc) as tc:
        with tc.tile_pool(name="persist", bufs=1) as pers:
            wq_t = pers.tile([PT, D], F32R)
            wk_t = pers.tile([PT, D], F32R)
            wv_t = pers.tile([PT, D], F32R)
            wo_t = pers.tile([PT, D], F32R)
            bq_t = pers.tile([PT, 1], F32)
            bk_t = pers.tile([PT, 1], F32)
            bv_t = pers.tile([1, PT], F32R)
            ones_t = pers.tile([1, PT], F32R)
            for dt_ in range(DT):
                ds = slice(dt_ * PT, (dt_ + 1) * PT)
                nc.sync.dma_start(wq_t[:, ds], wq_d[ds, :].bitcast(F32R))
                nc.sync.dma_start(wk_t[:, ds], wk_d[ds, :].bitcast(F32R))
                nc.sync.dma_start(wv_t[:, ds], wv_d[ds, :].bitcast(F32R))
            nc.sync.dma_start(wo_t[:], wo_d[:, :].bitcast(F32R))
            nc.sync.dma_start(bq_t[:], bq_d[:, :])
            nc.sync.dma_start(bk_t[:], bk_d[:, :])
            nc.sync.dma_start(bv_t[:], bv_d[:, :].bitcast(F32R))
            nc.sync.dma_start(ones_t[:], ones_d[:, :].bitcast(F32R))

            QhT = pers.tile([PT, S], F32R)   # [dout(2 heads), q]
            KhT = pers.tile([PT, S], F32R)   # [dout(2 heads), k]
            Vh = pers.tile([PT, S], F32)     # [k within tile, (ktile,dv2)]
            acc = pers.tile([PT, S], F32R)   # [dv(2 heads), q] attnV accum

            # ---- phase A: projections
            with (
                tc.tile_pool(name="xT", bufs=DT) as xp,
                tc.tile_pool(name="psA", bufs=2, space="PSUM") as psA,
            ):
                for xd, w_t, b_t, dst in (
                    (xq_d, wq_t, bq_t, QhT),
                    (xk_d, wk_t, bk_t, KhT),
                ):
                    xt = []
                    for dt_ in range(DT):
                        t = xp.tile([PT, S], F32R, tag="x")
                        nc.sync.dma_start(
                            t[:], xd[dt_ * PT:(dt_ + 1) * PT, :].bitcast(F32R))
                        xt.append(t)
                    for j in range(NQB):
                        qs = slice(j * QB, (j + 1) * QB)
                        ps = psA.tile([PT, QB], F32, tag="ps")
                        for dt_ in range(DT):
                            nc.tensor.matmul(
                                ps[:],
                                w_t[:, dt_ * PT:(dt_ + 1) * PT],
                                xt[dt_][:, qs],
                                start=(dt_ == 0), stop=(dt_ == DT - 1))
                        nc.vector.tensor_scalar_add(dst[:, qs], ps[:], b_t[:])

                xt = []
                for dt_ in range(DT):
                    t = xp.tile([PT, S], F32R, tag="x")
                    nc.sync.dma_start(
                        t[:], xv_d[dt_ * PT:(dt_ + 1) * PT, :].bitcast(F32R))
                    xt.append(t)
                for kt in range(KT):
                    ks = slice(kt * PT, (kt + 1) * PT)
                    pv = psA.tile([PT, PT], F32, tag="ps")
                    for dt_ in range(DT):
                        nc.tensor.matmul(
                            pv[:],
                            xt[dt_][:, ks],
                            wv_t[:, dt_ * PT:(dt_ + 1) * PT],
                            start=(dt_ == 0), stop=False)
                    nc.tensor.matmul(pv[:], ones_t[:], bv_t[:],
                                     start=False, stop=True)
                    nc.vector.tensor_copy(Vh[:, ks], pv[:])

            # ---- phase B: attention per head
            with (
                tc.tile_pool(name="exps", bufs=GRP + 2) as ep,
                tc.tile_pool(name="vhs", bufs=GRP + 2) as vp,
                tc.tile_pool(name="zp", bufs=4) as zp,
                tc.tile_pool(name="psB", bufs=2, space="PSUM") as psB,
                tc.tile_pool(name="psB2", bufs=2, space="PSUM") as psB2,
            ):
                for h in range(HPC):
                    hs = slice(h * DK, (h + 1) * DK)
                    et, vt = {}, {}
                    for kt in range(KT):
                        ks = slice(kt * PT, (kt + 1) * PT)
                        e = ep.tile([PT, S], F32R, tag="e")
                        zpt = zp.tile([PT, 4], F32, tag="zp")
                        for j in range(4):
                            ps = psB.tile([PT, 1024], F32, tag="s")
                            for half in range(2):
                                qo = j * 1024 + half * QB
                                nc.tensor.matmul(
                                    ps[:, half * QB:(half + 1) * QB],
                                    KhT[hs, ks],
                                    QhT[hs, qo:qo + QB],
                                    start=True, stop=True)
                            nc.scalar.activation(
                                e[:, j * 1024:(j + 1) * 1024], ps[:], EXP,
                                scale=0.125, accum_out=zpt[:, j:j + 1])
                        z = zp.tile([PT, 1], F32, tag="z")
                        nc.vector.reduce_sum(z[:], zpt[:], axis=AXX)
                        rz = zp.tile([PT, 1], F32, tag="rz")
                        nc.vector.reciprocal(rz[:], z[:])
                        vhs = vp.tile([PT, DK], F32R, tag="v")
                        nc.vector.tensor_scalar_mul(
                            vhs[:],
                            Vh[:, kt * PT + h * DK: kt * PT + (h + 1) * DK],
                            rz[:])
                        et[kt], vt[kt] = e, vhs

                        if kt % GRP == GRP - 1:
                            g0 = kt - GRP + 1
                            for j in range(NQB):
                                qs = slice(j * QB, (j + 1) * QB)
                                pa = psB2.tile([DK, QB], F32, tag="a")
                                for i in range(GRP):
                                    nc.tensor.matmul(
                                        pa[:], vt[g0 + i][:],
                                        et[g0 + i][:, qs],
                                        start=(i == 0), stop=(i == GRP - 1))
                                if g0 == 0:
                                    nc.vector.tensor_copy(acc[hs, qs], pa[:])
                                else:
                                    nc.vector.tensor_add(
                                        acc[hs, qs],
                                        acc[hs, qs].bitcast(F32), pa[:])
                            for i in range(GRP):
                                del et[g0 + i], vt[g0 + i]

            # ---- phase C: output projection
            with (
                tc.tile_pool(name="psC", bufs=2, space="PSUM") as psC,
                tc.tile_pool(name="ostage", bufs=3) as op,
            ):
                for dt_ in range(DT):
                    ds = slice(dt_ * PT, (dt_ + 1) * PT)
                    for j in range(NQB):
                        qs = slice(j * QB, (j + 1) * QB)
                        po = psC.tile([PT, QB], F32, tag="o")
                        nc.tensor.matmul(po[:], wo_t[:, ds], acc[:, qs],
                                         start=True, stop=True)
                        ot = op.tile([PT, QB], F32, tag="ot")
                        nc.vector.tensor_copy(ot[:], po[:])
                        nc.sync.dma_start(out_d[ds, qs], ot[:])

    nc.compile()
    return nc


def _in_maps(q, k, v, Wq, bq, Wk, bk, Wv, bv, Wo):
    maps = []
    for c in range(NCORES):
        b, hp = divmod(c, NCORES // B)
        cs = slice(hp * PT, (hp + 1) * PT)
        maps.append({
            "xq": np.ascontiguousarray(q[b].T, dtype=np.float32),
            "xk": np.ascontiguousarray(k[b].T, dtype=np.float32),
            "xv": np.ascontiguousarray(v[b].T, dtype=np.float32),
            "wq": np.ascontiguousarray(Wq[:, cs], dtype=np.float32),
            "wk": np.ascontiguousarray(Wk[:, cs], dtype=np.float32),
            "wv": np.ascontiguousarray(Wv[:, cs], dtype=np.float32),
            "wo": np.ascontiguousarray(Wo[cs, :], dtype=np.float32),
            "bq": np.ascontiguousarray(bq[cs].reshape(PT, 1), dtype=np.float32),
            "bk": np.ascontiguousarray(bk[cs].reshape(PT, 1), dtype=np.float32),
            "bv": np.ascontiguousarray(bv[cs].reshape(1, PT), dtype=np.float32),
            "ones": np.ones((1, PT), dtype=np.float32),
        })
    return maps


def _run(q, k, v, Wq, bq, Wk, bk, Wv, bv, Wo, bo, **spmd_kwargs):
    global _cached_nc
    if _cached_nc is None:
        _cached_nc = _build()
    maps = _in_maps(q, k, v, Wq, bq, Wk, bk, Wv, bv, Wo)
    res = run_bass_kernel_spmd(_cached_nc, maps,
                               core_ids=list(range(NCORES)), **spmd_kwargs)
    out = np.zeros((B, S, D), np.float32)
    for c in range(NCORES):
        b = c // (NCORES // B)
        out[b] += np.asarray(res.results[c]["outT"]).T
    out += np.asarray(bo, dtype=np.float32)
    return out, res


def kernel(q, k, v, Wq, bq, Wk, bk, Wv, bv, Wo, bo):
    out, _ = _run(q, k, v, Wq, bq, Wk, bk, Wv, bv, Wo, bo)
    return out
